# revision 1
# baseline (speedup 1.0000x reference)
"""CombinedCSA (channel+spatial attention) Trainium2 Bass kernel.

Sharding: data-parallel over batch. 16 images / 8 cores = 2 images per core.
Weights (fc1/fc2/conv) replicated, pre-transposed host-side.

Per-core dataflow (per image, streamed in HW chunks of 16 rows):
  load chunk -> channel-max (DVE reduce) + channel-sum (ACT accum_out)
  MLP (PE matmuls + ACT relu/sigmoid) -> per-channel scale
  scale chunk in place (ACT, per-partition scale)
  spatial max over C: DVE max(half0,half1) -> PE transpose -> DVE segmented reduce
  spatial sum over C: PE matmul (x block stationary, ones moving)
  7x7 conv: 14 banded matmuls on PE (bands built host-side)
  sigmoid -> transpose -> row-collapse DMA -> gpsimd partition_broadcast
  final multiply in place (DVE / gpsimd split) -> store
"""

import os
import numpy as np
from contextlib import ExitStack

import concourse.bass as bass
import concourse.tile as tile
from concourse import bacc, mybir
from concourse._compat import with_exitstack
from concourse.bass_utils import run_bass_kernel_spmd

F32 = mybir.dt.float32
AF = mybir.ActivationFunctionType

# Problem constants (hardcoded; see spec)
B, C, H, W = 16, 256, 128, 128
HW = H * W          # 16384
R = 16              # Cr = C // 16
NCORES = 8
BLOC = B // NCORES  # 2 images per core
NH = 2              # channel halves of 128
P = 128
FCH = 2048          # hw elements per chunk (16 h-rows)
NCH = HW // FCH     # 8 chunks per image
HROWS = FCH // W    # 16 h-rows per chunk
CONVG = 2           # chunks per conv group
NBLK = FCH // P     # 16 transpose blocks per chunk

# chunk indices whose heavy elementwise ops go to gpsimd instead of DVE
# (walrus rejects TensorTensor on the Pool engine on this toolchain, so empty)
GPS_FINAL = frozenset()
GPS_COMBINE = frozenset()


@with_exitstack
def csa_kernel(ctx, tc, out_d, x_d, w1t_d, w2t_d, bands_d, ident_d,
               skip=frozenset()):
    nc = tc.nc

    # ---- pools ----
    xp = ctx.enter_context(tc.tile_pool(name="xp", bufs=19))
    xmaxp = ctx.enter_context(tc.tile_pool(name="xmaxp", bufs=2))
    bcp = ctx.enter_context(tc.tile_pool(name="bcp", bufs=1))
    rowp = ctx.enter_context(tc.tile_pool(name="rowp", bufs=1))
    stat = ctx.enter_context(tc.tile_pool(name="stat", bufs=2))
    cons = ctx.enter_context(tc.tile_pool(name="cons", bufs=1))
    tp = ctx.enter_context(tc.tile_pool(name="tp", bufs=2, space="PSUM"))
    tsp = ctx.enter_context(tc.tile_pool(name="tsp", bufs=2, space="PSUM"))
    convp = ctx.enter_context(tc.tile_pool(name="convp", bufs=2, space="PSUM"))
    atpp = ctx.enter_context(tc.tile_pool(name="atpp", bufs=1, space="PSUM"))
    mlpp = ctx.enter_context(tc.tile_pool(name="mlpp", bufs=1, space="PSUM"))

    # ---- constants / weights ----
    w1t_sb = cons.tile([P, NH * R], F32)           # [128, 32]: col block h = w_fc1.T half h
    for h in range(NH):
        nc.sync.dma_start(out=w1t_sb[:, h * R:(h + 1) * R],
                          in_=w1t_d[h * P:(h + 1) * P, :])
    w2t_sb = cons.tile([R, C], F32)                # [16, 256] = w_fc2.T
    nc.sync.dma_start(out=w2t_sb[:], in_=w2t_d[:])
    bands_sb = cons.tile([P, 14 * P], F32)         # [128, (ci, w)]
    nc.sync.dma_start(out=bands_sb[:].rearrange("p (c w) -> p c w", c=14),
                      in_=bands_d.transpose([1, 0, 2]))
    ident_sb = cons.tile([P, P], F32)
    nc.sync.dma_start(out=ident_sb[:], in_=ident_d[:])
    ones_sb = cons.tile([P, 1], F32)
    nc.vector.memset(ones_sb[:], 1.0)

    for b in range(BLOC):
        # ---------- phase A: load + channel pooling ----------
        xt = [[None] * NCH for _ in range(NH)]
        chmax_p = []
        chsum_p = []
        for h in range(NH):
            cmp_t = stat.tile([P, NCH], F32, name=f"chmaxp{b}{h}", tag=f"chmaxp{h}")
            csp_t = stat.tile([P, NCH], F32, name=f"chsump{b}{h}", tag=f"chsump{h}")
            chmax_p.append(cmp_t)
            chsum_p.append(csp_t)
            if "chpool" in skip:
                nc.vector.memset(cmp_t[:], 0.5)
                nc.vector.memset(csp_t[:], 0.5)
        for k in range(NCH):
            for h in range(NH):
                t = xp.tile([P, FCH], F32, name=f"x{b}{h}{k}", tag="x")
                xt[h][k] = t
                nc.sync.dma_start(
                    out=t[:],
                    in_=x_d[b, h * P:(h + 1) * P, k * FCH:(k + 1) * FCH])
                if "chpool" in skip:
                    continue
                nc.vector.tensor_reduce(
                    out=chmax_p[h][:, k:k + 1], in_=t[:],
                    axis=mybir.AxisListType.X, op=mybir.AluOpType.max)
                # in-place copy whose only purpose is the free-dim sum output
                nc.scalar.activation(
                    out=t[:], in_=t[:], func=AF.Copy,
                    accum_out=chsum_p[h][:, k:k + 1])

        # ---------- phase B: channel-attention MLP ----------
        scale_sb = []
        z_ps = mlpp.tile([R, 1], F32, name=f"zps{b}", tag="mlp")
        hvec = []
        for h in range(NH):
            cmf = stat.tile([P, 1], F32, name=f"chmaxf{b}{h}", tag=f"chmaxf{h}")
            csf = stat.tile([P, 1], F32, name=f"chsumf{b}{h}", tag=f"chsumf{h}")
            nc.vector.tensor_reduce(out=cmf[:], in_=chmax_p[h][:],
                                    axis=mybir.AxisListType.X,
                                    op=mybir.AluOpType.max)
            nc.vector.tensor_reduce(out=csf[:], in_=chsum_p[h][:],
                                    axis=mybir.AxisListType.X,
                                    op=mybir.AluOpType.add)
            hv = stat.tile([P, 1], F32, name=f"hvec{b}{h}", tag=f"hvec{h}")
            # hv = chmax + chsum/HW
            nc.scalar.activation(out=hv[:], in_=csf[:], func=AF.Identity,
                                 bias=cmf[:, 0:1], scale=1.0 / HW)
            hvec.append(hv)
        for h in range(NH):
            nc.tensor.matmul(out=z_ps[:], lhsT=w1t_sb[:, h * R:(h + 1) * R],
                             rhs=hvec[h][:], start=(h == 0), stop=(h == NH - 1))
        zr = stat.tile([R, 1], F32, name=f"zrelu{b}", tag="zrelu")
        nc.scalar.activation(out=zr[:], in_=z_ps[:], func=AF.Relu)
        for h in range(NH):
            l_ps = mlpp.tile([P, 1], F32, name=f"lps{b}{h}", tag="mlp")
            nc.tensor.matmul(out=l_ps[:], lhsT=w2t_sb[:, h * P:(h + 1) * P],
                             rhs=zr[:], start=True, stop=True)
            sc = stat.tile([P, 1], F32, name=f"scale{b}{h}", tag=f"scale{h}")
            nc.scalar.activation(out=sc[:], in_=l_ps[:], func=AF.Sigmoid)
            scale_sb.append(sc)

        # ---------- phase C/D/E: scale, spatial stats, conv, final ----------
        smaxT = stat.tile([P, H], F32, name=f"smaxT{b}", tag="smaxT")   # [w, h]
        savgT = stat.tile([P, H], F32, name=f"savgT{b}", tag="savgT")   # [w, h]
        conv_ps = convp.tile([P, H], F32, name=f"convps{b}", tag="conv")
        bcs = {}
        if "trans" in skip:
            nc.vector.memset(smaxT[:], 0.25)
        if "savg" in skip:
            nc.vector.memset(savgT[:], 0.25)

        def conv_pair(g):
            h0c, h1c = g * CONVG * HROWS, (g + 1) * CONVG * HROWS
            # 7x7 conv as banded matmuls: out[:, h] += bandT_{c,i} @ statT[:, h+i-3]
            mms = []
            for c, st in ((0, smaxT), (1, savgT)):
                for i in range(7):
                    lo = max(h0c, 3 - i)
                    hi = min(h1c, H + 3 - i)
                    if lo >= hi:
                        continue
                    mms.append((c, i, lo, hi, st))
            # identity-shift tap first so start=True covers the whole column range
            mms.sort(key=lambda m: (m[1] != 3 or m[0] != 0))
            for n, (c, i, lo, hi, st) in enumerate(mms):
                assert not (n == 0 and (lo != h0c or hi != h1c))
                nc.tensor.matmul(
                    out=conv_ps[:, lo:hi],
                    lhsT=bands_sb[:, (c * 7 + i) * P:(c * 7 + i + 1) * P],
                    rhs=st[:, lo + i - 3:hi + i - 3],
                    start=(n == 0), stop=(n == len(mms) - 1),
                    skip_group_check=True)

        def attn_chunk(kc):
            h0c, h1c = kc * HROWS, (kc + 1) * HROWS
            attn_wh = stat.tile([P, HROWS], F32, name=f"attnwh{b}{kc}",
                                tag="attnwh", bufs=3)
            nc.scalar.activation(out=attn_wh[:], in_=conv_ps[:, h0c:h1c],
                                 func=AF.Sigmoid)
            at_ps = atpp.tile([HROWS, P], F32, name=f"atps{b}{kc}", tag="atp")
            nc.tensor.transpose(out=at_ps[:], in_=attn_wh[:], identity=ident_sb[:])
            attn_hw = stat.tile([HROWS, P], F32, name=f"attnhw{b}{kc}",
                                tag="attnhw", bufs=3)
            nc.scalar.activation(out=attn_hw[:], in_=at_ps[:], func=AF.Copy)
            row = rowp.tile([1, FCH], F32, name=f"row{b}{kc}", tag="row")
            nc.sync.dma_start(
                out=row[:].rearrange("p (h w) -> p h w", h=HROWS),
                in_=attn_hw[:])
            bc = bcp.tile([P, FCH], F32, name=f"bc{b}{kc}", tag="bc")
            nc.gpsimd.partition_broadcast(bc[:], row[:], channels=P)
            bcs[kc] = bc

        def conv_and_final(g):
            if "conv" not in skip:
                conv_pair(g)
            for kc in range(CONVG * g, CONVG * (g + 1)):
                if "conv" not in skip:
                    attn_chunk(kc)
                for h in range(NH):
                    if "final" not in skip and "conv" not in skip:
                        nc.vector.tensor_mul(xt[h][kc][:], xt[h][kc][:],
                                             bcs[kc][:])
                    nc.sync.dma_start(
                        out=out_d[b, h * P:(h + 1) * P,
                                  kc * FCH:(kc + 1) * FCH],
                        in_=xt[h][kc][:])

        for k in range(NCH):
            if "scale" not in skip:
                for h in range(NH):
                    nc.scalar.activation(out=xt[h][k][:], in_=xt[h][k][:],
                                         func=AF.Copy,
                                         scale=scale_sb[h][:, 0:1])
            # spatial max over C: combine halves, transpose, segmented reduce.
            # spatial sum over C: transpose both halves into the same PSUM
            # region with accumulation, then segmented add-reduce.
            if "trans" not in skip:
                xm = xmaxp.tile([P, FCH], F32, name=f"xm{b}{k}", tag="xm")
                nc.vector.tensor_max(xm[:], xt[0][k][:], xt[1][k][:])
                for j4 in range(NBLK // 4):
                    tpt = tp.tile([P, 4 * P], F32, name=f"tp{b}{k}{j4}",
                                  tag="tp")
                    tps = tsp.tile([P, 4 * P], F32, name=f"ts{b}{k}{j4}",
                                   tag="ts")
                    for jj in range(4):
                        j = j4 * 4 + jj
                        nc.tensor.transpose(out=tpt[:, jj * P:(jj + 1) * P],
                                            in_=xm[:, j * P:(j + 1) * P],
                                            identity=ident_sb[:])
                        if "savg" in skip:
                            continue
                        nc.tensor.matmul(out=tps[:, jj * P:(jj + 1) * P],
                                         lhsT=xt[0][k][:, j * P:(j + 1) * P],
                                         rhs=ident_sb[:], is_transpose=True,
                                         start=True, stop=False,
                                         skip_group_check=True)
                        nc.tensor.matmul(out=tps[:, jj * P:(jj + 1) * P],
                                         lhsT=xt[1][k][:, j * P:(j + 1) * P],
                                         rhs=ident_sb[:], is_transpose=True,
                                         start=False, stop=True,
                                         skip_group_check=True)
                    g0 = k * NBLK + j4 * 4
                    nc.vector.tensor_reduce(
                        out=smaxT[:, g0:g0 + 4],
                        in_=tpt[:].rearrange("p (b f) -> p b f", b=4),
                        axis=mybir.AxisListType.X, op=mybir.AluOpType.max)
                    if "savg" not in skip:
                        nc.vector.tensor_reduce(
                            out=savgT[:, g0:g0 + 4],
                            in_=tps[:].rearrange("p (b f) -> p b f", b=4),
                            axis=mybir.AxisListType.X, op=mybir.AluOpType.add)
            if k >= CONVG and k % CONVG == 0:
                conv_and_final((k - CONVG) // CONVG)
        conv_and_final(NCH // CONVG - 1)


def _build_nc(reps: int = 1, skip=frozenset()):
    nc = bacc.Bacc("TRN2", target_bir_lowering=False, debug=False,
                   num_devices=NCORES)
    x_d = nc.dram_tensor("x", [BLOC, C, HW], F32, kind="ExternalInput").ap()
    w1t_d = nc.dram_tensor("w1t", [C, R], F32, kind="ExternalInput").ap()
    w2t_d = nc.dram_tensor("w2t", [R, C], F32, kind="ExternalInput").ap()
    bands_d = nc.dram_tensor("bands", [14, W, W], F32, kind="ExternalInput").ap()
    ident_d = nc.dram_tensor("ident", [P, P], F32, kind="ExternalInput").ap()
    out_d = nc.dram_tensor("out", [BLOC, C, HW], F32, kind="ExternalOutput").ap()
    with tile.TileContext(nc) as tc:
        for _ in range(reps):
            csa_kernel(tc, out_d, x_d, w1t_d, w2t_d, bands_d, ident_d,
                       skip=skip)
    nc.compile()
    return nc


_NC_CACHE = None


def _get_nc():
    global _NC_CACHE
    if _NC_CACHE is None:
        _NC_CACHE = _build_nc()
    return _NC_CACHE


def build_bands(w_conv):
    """[14, W, W] transposed band matrices; bands[c*7+i][w', w] =
    w_conv[0, c, i, w'-w+3]; avg channel folded with 1/C."""
    w_conv = np.asarray(w_conv, np.float32)
    bands = np.zeros((2, 7, W, W), np.float32)
    for c in range(2):
        for i in range(7):
            for kj in range(7):
                bands[c, i] += w_conv[0, c, i, kj] * np.eye(W, k=3 - kj,
                                                            dtype=np.float32)
    bands[1] /= C
    return bands.reshape(14, W, W)


def make_in_maps(x, w_fc1, w_fc2, w_conv):
    x = np.ascontiguousarray(np.asarray(x, np.float32))
    w1t = np.ascontiguousarray(np.asarray(w_fc1, np.float32).T)
    w2t = np.ascontiguousarray(np.asarray(w_fc2, np.float32).T)
    bands = build_bands(w_conv)
    ident = np.eye(P, dtype=np.float32)
    xr = x.reshape(NCORES, BLOC, C, HW)
    return [{"x": np.ascontiguousarray(xr[i]), "w1t": w1t, "w2t": w2t,
             "bands": bands, "ident": ident} for i in range(NCORES)]


def kernel(x, w_fc1, w_fc2, w_conv):
    nc = _get_nc()
    in_maps = make_in_maps(x, w_fc1, w_fc2, w_conv)
    res = run_bass_kernel_spmd(nc, in_maps, list(range(NCORES)))
    out = np.stack([res.results[i]["out"] for i in range(NCORES)])
    return out.reshape(B, C, H, W).astype(np.float32)



# revision 14
# speedup vs baseline: 1.7032x; 1.7032x over previous
"""CombinedCSA (channel+spatial attention) Trainium2 Bass kernel, fp16.

Sharding: data-parallel over batch. 16 images / 8 cores = 2 images per core.
Weights (fc1/fc2/conv) replicated, pre-transposed host-side. x is cast to
fp16 host-side (tolerance is 2e-2; fp16 keeps rel err ~1e-3) which halves
HBM traffic — the memory roofline — and doubles DVE tensor-tensor speed.

HW-measured op costs ([128,4096] fp16 unless noted) drove engine choices:
  DVE tensor_scalar 0.8us (4x) / TT 1.1-1.8us (2x) / STT-with-accum ~1us
  / tensor_reduce 4.1us (1x, unavoidable);  ACT copy+accum ~3us;
  Pool broadcast 5.1us, partition_all_reduce 12.8us (too slow — unused);
  PE back-to-back single-column matmuls ~17ns (savg is nearly free).
  tensor_tensor_reduce and DVE pool_max crash the device — banned.

Per-core dataflow (per image, chunks of 32 h-rows = [128, 4096] fp16):
  load chunk       -> chmax: DVE TT-max fold tree + short reduce
                      chsum: ACT in-place copy w/ accum_out (parallel engine)
  MLP              -> hv/relu on DVE, matmuls PE, sigmoid ACT -> scale s[h]
  scale            -> DVE tensor_scalar in place (4x mode)
  spatial max      -> TT max halves -> PE transpose (fp16 PSUM) -> ACT copy
                      to SBUF -> DVE in-block max tree + segmented reduce
  spatial sum      -> PE matmuls: x block stationary, ones moving (n=1)
  7x7 conv         -> 14 banded matmuls on PE (bands built host-side)
  sigmoid -> transpose -> row-collapse DMA -> gpsimd partition_broadcast
  final            -> out = xs * attn_bc (DVE TT), store
"""

import numpy as np
from contextlib import ExitStack

import concourse.bass as bass
import concourse.tile as tile
from concourse import bacc, mybir, bass_isa
from concourse._compat import with_exitstack
from concourse.bass_utils import run_bass_kernel_spmd

F32 = mybir.dt.float32
F16 = mybir.dt.float16
AF = mybir.ActivationFunctionType
ALU = mybir.AluOpType

# Problem constants (hardcoded; see spec)
B, C, H, W = 16, 256, 128, 128
HW = H * W          # 16384
R = 16              # Cr = C // 16
NCORES = 8
BLOC = B // NCORES  # 2 images per core
NH = 2              # channel halves of 128
P = 128
FCH = 4096          # hw elements per chunk (32 h-rows)
NCH = HW // FCH     # 4 chunks per image
HROWS = FCH // W    # 32 h-rows per chunk
NBLK = FCH // P     # 32 w-blocks (h-rows) per chunk
HT = 16             # h-rows per transposed PSUM tile (2 tiles per chunk)


@with_exitstack
def csa_kernel(ctx, tc, out_d, x_d, w1t_d, w2t_d, bands_d, ident_d,
               skip=frozenset()):
    nc = tc.nc

    # ---- pools ----
    xp = ctx.enter_context(tc.tile_pool(name="xp", bufs=16))    # x chunks 1MiB
    m1p = ctx.enter_context(tc.tile_pool(name="m1p", bufs=2))   # chmax fold
    m2p = ctx.enter_context(tc.tile_pool(name="m2p", bufs=2))
    sp = ctx.enter_context(tc.tile_pool(name="sp", bufs=2))     # xm
    stp = ctx.enter_context(tc.tile_pool(name="stp", bufs=2))   # smax tree
    bcp = ctx.enter_context(tc.tile_pool(name="bcp", bufs=2))   # attn bcast
    rowp = ctx.enter_context(tc.tile_pool(name="rowp", bufs=1))
    stat = ctx.enter_context(tc.tile_pool(name="stat", bufs=2))
    cons = ctx.enter_context(tc.tile_pool(name="cons", bufs=1))
    psT = ctx.enter_context(tc.tile_pool(name="psT", bufs=2, space="PSUM"))
    psS = ctx.enter_context(tc.tile_pool(name="psS", bufs=1, space="PSUM"))
    psC = ctx.enter_context(tc.tile_pool(name="psC", bufs=1, space="PSUM"))
    psA = ctx.enter_context(tc.tile_pool(name="psA", bufs=1, space="PSUM"))
    psM = ctx.enter_context(tc.tile_pool(name="psM", bufs=1, space="PSUM"))

    # ---- constants / weights ----
    w1t_sb = cons.tile([P, NH * R], F32)           # [128, 32]: col block h = w_fc1.T half h
    for h in range(NH):
        nc.sync.dma_start(out=w1t_sb[:, h * R:(h + 1) * R],
                          in_=w1t_d[h * P:(h + 1) * P, :])
    w2t_sb = cons.tile([R, C], F32)                # [16, 256] = w_fc2.T
    nc.sync.dma_start(out=w2t_sb[:], in_=w2t_d[:])
    bands_sb = cons.tile([P, 14 * P], F16)         # [128, (ci, w)]
    nc.sync.dma_start(out=bands_sb[:].rearrange("p (c w) -> p c w", c=14),
                      in_=bands_d.transpose([1, 0, 2]))
    ident_sb = cons.tile([P, P], F16)
    nc.sync.dma_start(out=ident_sb[:], in_=ident_d[:])
    ones16 = cons.tile([P, 1], F16)
    nc.vector.memset(ones16[:], 1.0)

    for b in range(BLOC):
        # ---------- phase A: load + channel pooling ----------
        xt = [[None] * NCH for _ in range(NH)]
        chmax_p = []
        chsum_p = []
        for h in range(NH):
            cmp_t = stat.tile([P, NCH], F32, name=f"chmaxp{b}{h}", tag=f"chmaxp{h}")
            csp_t = stat.tile([P, NCH], F32, name=f"chsump{b}{h}", tag=f"chsump{h}")
            chmax_p.append(cmp_t)
            chsum_p.append(csp_t)
            if "chpool" in skip:
                nc.vector.memset(cmp_t[:], 0.5)
                nc.vector.memset(csp_t[:], 0.5)
        for k in range(NCH):
            for h in range(NH):
                t = xp.tile([P, FCH], F16, name=f"x{b}{h}{k}", tag="x")
                xt[h][k] = t
                nc.sync.dma_start(
                    out=t[:],
                    in_=x_d[b, h * P:(h + 1) * P, k * FCH:(k + 1) * FCH])
                if "chpool" in skip:
                    continue
                # chmax: two TT-max fold levels then a short 1x reduce
                m1 = m1p.tile([P, FCH // 2], F16, name=f"m1{b}{h}{k}", tag="m1")
                nc.vector.tensor_max(m1[:], t[:, :FCH // 2], t[:, FCH // 2:])
                m2 = m2p.tile([P, FCH // 4], F16, name=f"m2{b}{h}{k}", tag="m2")
                nc.vector.tensor_max(m2[:], m1[:, :FCH // 4], m1[:, FCH // 4:])
                nc.vector.tensor_reduce(
                    out=chmax_p[h][:, k:k + 1], in_=m2[:],
                    axis=mybir.AxisListType.X, op=ALU.max)
                # in-place copy whose only purpose is the free-dim sum output
                nc.scalar.activation(
                    out=t[:], in_=t[:], func=AF.Copy,
                    accum_out=chsum_p[h][:, k:k + 1])

        # ---------- phase B: channel-attention MLP ----------
        scale_f = []     # f32 [128, 1] per half (tensor_scalar operand)
        z_ps = psM.tile([R, 1], F32, name=f"zps{b}", tag="mlp")
        hvec = []
        for h in range(NH):
            cmf = stat.tile([P, 1], F32, name=f"chmaxf{b}{h}", tag=f"chmaxf{h}")
            csf = stat.tile([P, 1], F32, name=f"chsumf{b}{h}", tag=f"chsumf{h}")
            nc.vector.tensor_reduce(out=cmf[:], in_=chmax_p[h][:],
                                    axis=mybir.AxisListType.X,
                                    op=ALU.max)
            nc.vector.tensor_reduce(out=csf[:], in_=chsum_p[h][:],
                                    axis=mybir.AxisListType.X,
                                    op=ALU.add)
            hv = stat.tile([P, 1], F32, name=f"hvec{b}{h}", tag=f"hvec{h}")
            # hv = chmax + chsum/HW  (on DVE to keep the MLP off the busy ACT)
            nc.vector.scalar_tensor_tensor(
                out=hv[:], in0=csf[:], scalar=1.0 / HW, in1=cmf[:],
                op0=ALU.mult, op1=ALU.add)
            hvec.append(hv)
        for h in range(NH):
            nc.tensor.matmul(out=z_ps[:], lhsT=w1t_sb[:, h * R:(h + 1) * R],
                             rhs=hvec[h][:], start=(h == 0), stop=(h == NH - 1))
        zr = stat.tile([R, 1], F32, name=f"zrelu{b}", tag="zrelu")
        nc.vector.tensor_scalar_max(zr[:], z_ps[:], 0.0)
        for h in range(NH):
            l_ps = psM.tile([P, 1], F32, name=f"lps{b}{h}", tag="mlp")
            nc.tensor.matmul(out=l_ps[:], lhsT=w2t_sb[:, h * P:(h + 1) * P],
                             rhs=zr[:], start=True, stop=True)
            sc = stat.tile([P, 1], F32, name=f"scale{b}{h}", tag=f"scale{h}")
            nc.scalar.activation(out=sc[:], in_=l_ps[:], func=AF.Sigmoid)
            scale_f.append(sc)

        # ---------- phase C/D/E: spatial stats, conv, final ----------
        smaxT = stat.tile([P, H], F16, name=f"smaxT{b}", tag="smaxT")   # [w, h]
        savgT = stat.tile([P, H], F16, name=f"savgT{b}", tag="savgT")   # [w, h]
        conv_ps = psC.tile([P, H], F32, name=f"convps{b}", tag="conv")
        if "smax" in skip:
            nc.vector.memset(smaxT[:], 0.25)
        if "savg" in skip:
            nc.vector.memset(savgT[:], 0.25)

        def stats_chunk(k):
            # apply the per-channel scale to x in place (4x-mode tensor_scalar)
            if "scale" not in skip:
                for h in range(NH):
                    nc.vector.tensor_scalar_mul(xt[h][k][:], xt[h][k][:],
                                                scale_f[h][:, 0:1])
            # spatial max over C: TT-max the halves, PE-transpose 128-blocks
            # into fp16 PSUM, copy to SBUF (ACT), then an in-block max tree +
            # segmented reduce on DVE -> smaxT[w, h] columns
            if "smax" not in skip:
                xm = sp.tile([P, FCH], F16, name=f"xm{b}{k}", tag="sp")
                nc.vector.tensor_max(xm[:], xt[0][k][:], xt[1][k][:])
                for half in range(2):
                    tp_ps = psT.tile([P, HT * P], F16,
                                     name=f"tps{b}{k}{half}", tag="tp")
                    for j in range(HT):
                        jj = half * HT + j
                        nc.tensor.transpose(
                            out=tp_ps[:, j * P:(j + 1) * P],
                            in_=xm[:, jj * P:(jj + 1) * P],
                            identity=ident_sb[:])
                    sT = stp.tile([P, HT * P], F16, name=f"sT{b}{k}{half}",
                                  tag="sT")
                    nc.scalar.activation(out=sT[:], in_=tp_ps[:], func=AF.Copy)
                    v = sT[:].rearrange("p (h c) -> p h c", h=HT)
                    l1 = stp.tile([P, HT * 64], F16, name=f"l1{b}{k}{half}",
                                  tag="l1")
                    v1 = l1[:].rearrange("p (h c) -> p h c", h=HT)
                    nc.vector.tensor_max(v1[:], v[:, :, 0:64], v[:, :, 64:128])
                    l2 = stp.tile([P, HT * 32], F16, name=f"l2{b}{k}{half}",
                                  tag="l2")
                    v2 = l2[:].rearrange("p (h c) -> p h c", h=HT)
                    nc.vector.tensor_max(v2[:], v1[:, :, 0:32], v1[:, :, 32:64])
                    c0 = k * HROWS + half * HT
                    nc.vector.tensor_reduce(
                        out=smaxT[:, c0:c0 + HT], in_=v2[:],
                        axis=mybir.AxisListType.X, op=ALU.max)
            # spatial (scaled) sum over C: per h-row, PE matmul with the x
            # block stationary and the ones vector moving; halves accumulate
            # in PSUM. avg's 1/C is folded into the conv bands.
            if "savg" not in skip:
                sps = psS.tile([P, HROWS], F32, name=f"sps{b}{k}", tag="savg")
                for j in range(NBLK):
                    nc.tensor.matmul(
                        out=sps[:, j:j + 1],
                        lhsT=xt[0][k][:, j * P:(j + 1) * P],
                        rhs=ones16[:], start=True, stop=False,
                        skip_group_check=True)
                    nc.tensor.matmul(
                        out=sps[:, j:j + 1],
                        lhsT=xt[1][k][:, j * P:(j + 1) * P],
                        rhs=ones16[:], start=False, stop=True,
                        skip_group_check=True)
                nc.scalar.activation(
                    out=savgT[:, k * HROWS:(k + 1) * HROWS], in_=sps[:],
                    func=AF.Copy)

        def conv_chunk(g):
            h0c, h1c = g * HROWS, (g + 1) * HROWS
            if "conv" not in skip:
                # 7x7 conv as banded matmuls: out[:, h] += bandT_{c,i} @ statT[:, h+i-3]
                mms = []
                for c, st in ((0, smaxT), (1, savgT)):
                    for i in range(7):
                        lo = max(h0c, 3 - i)
                        hi = min(h1c, H + 3 - i)
                        if lo >= hi:
                            continue
                        mms.append((c, i, lo, hi, st))
                # identity-shift tap first so start=True covers the whole column range
                mms.sort(key=lambda m: (m[1] != 3 or m[0] != 0))
                for n, (c, i, lo, hi, st) in enumerate(mms):
                    assert not (n == 0 and (lo != h0c or hi != h1c))
                    nc.tensor.matmul(
                        out=conv_ps[:, lo:hi],
                        lhsT=bands_sb[:, (c * 7 + i) * P:(c * 7 + i + 1) * P],
                        rhs=st[:, lo + i - 3:hi + i - 3],
                        start=(n == 0), stop=(n == len(mms) - 1),
                        skip_group_check=True)
                attn_wh = stat.tile([P, HROWS], F16, name=f"attnwh{b}{g}",
                                    tag="attnwh")
                nc.scalar.activation(out=attn_wh[:], in_=conv_ps[:, h0c:h1c],
                                     func=AF.Sigmoid)
                at_ps = psA.tile([HROWS, P], F16, name=f"atps{b}{g}", tag="atp")
                nc.tensor.transpose(out=at_ps[:], in_=attn_wh[:],
                                    identity=ident_sb[:])
                attn_hw = stat.tile([HROWS, P], F16, name=f"attnhw{b}{g}",
                                    tag="attnhw")
                nc.scalar.activation(out=attn_hw[:], in_=at_ps[:], func=AF.Copy)
                arow = rowp.tile([1, FCH], F16, name=f"arow{b}{g}", tag="arow")
                nc.sync.dma_start(
                    out=arow[:].rearrange("p (h w) -> p h w", h=HROWS),
                    in_=attn_hw[:])
                bc = bcp.tile([P, FCH], F16, name=f"bc{b}{g}", tag="bc")
                nc.gpsimd.partition_broadcast(bc[:], arow[:], channels=P)
            for h in range(NH):
                if "final" not in skip and "conv" not in skip:
                    nc.vector.tensor_mul(xt[h][g][:], xt[h][g][:], bc[:])
                nc.sync.dma_start(
                    out=out_d[b, h * P:(h + 1) * P, g * FCH:(g + 1) * FCH],
                    in_=xt[h][g][:])

        for k in range(NCH):
            stats_chunk(k)
            if k >= 1:
                conv_chunk(k - 1)
        conv_chunk(NCH - 1)


def _build_nc(reps: int = 1, skip=frozenset()):
    nc = bacc.Bacc("TRN2", target_bir_lowering=False, debug=False,
                   num_devices=NCORES)
    x_d = nc.dram_tensor("x", [BLOC, C, HW], F16, kind="ExternalInput").ap()
    w1t_d = nc.dram_tensor("w1t", [C, R], F32, kind="ExternalInput").ap()
    w2t_d = nc.dram_tensor("w2t", [R, C], F32, kind="ExternalInput").ap()
    bands_d = nc.dram_tensor("bands", [14, W, W], F16, kind="ExternalInput").ap()
    ident_d = nc.dram_tensor("ident", [P, P], F16, kind="ExternalInput").ap()
    out_d = nc.dram_tensor("out", [BLOC, C, HW], F16, kind="ExternalOutput").ap()
    with tile.TileContext(nc) as tc:
        for _ in range(reps):
            csa_kernel(tc, out_d, x_d, w1t_d, w2t_d, bands_d, ident_d,
                       skip=skip)
    nc.compile()
    return nc


_NC_CACHE = None


def _get_nc():
    global _NC_CACHE
    if _NC_CACHE is None:
        _NC_CACHE = _build_nc()
    return _NC_CACHE


def build_bands(w_conv):
    """[14, W, W] transposed band matrices; bands[c*7+i][w', w] =
    w_conv[0, c, i, w'-w+3]; avg channel folded with 1/C."""
    w_conv = np.asarray(w_conv, np.float32)
    bands = np.zeros((2, 7, W, W), np.float32)
    for c in range(2):
        for i in range(7):
            for kj in range(7):
                bands[c, i] += w_conv[0, c, i, kj] * np.eye(W, k=3 - kj,
                                                            dtype=np.float32)
    bands[1] /= C
    return bands.reshape(14, W, W).astype(np.float16)


def make_in_maps(x, w_fc1, w_fc2, w_conv):
    x16 = np.ascontiguousarray(np.asarray(x, np.float32).astype(np.float16))
    w1t = np.ascontiguousarray(np.asarray(w_fc1, np.float32).T)
    w2t = np.ascontiguousarray(np.asarray(w_fc2, np.float32).T)
    bands = build_bands(w_conv)
    ident = np.eye(P, dtype=np.float16)
    xr = x16.reshape(NCORES, BLOC, C, HW)
    return [{"x": np.ascontiguousarray(xr[i]), "w1t": w1t, "w2t": w2t,
             "bands": bands, "ident": ident} for i in range(NCORES)]


def kernel(x, w_fc1, w_fc2, w_conv):
    nc = _get_nc()
    in_maps = make_in_maps(x, w_fc1, w_fc2, w_conv)
    res = run_bass_kernel_spmd(nc, in_maps, list(range(NCORES)))
    out = np.stack([res.results[i]["out"] for i in range(NCORES)])
    return out.reshape(B, C, H, W).astype(np.float32)
